# revision 2
# baseline (speedup 1.0000x reference)
"""Trainium2 Bass kernel v2 for nn_MultiHeadAttention (channel-attention block).

Math per batch (X* = reshape(*, [C,P]), P=4096, C=128, D=512, 8 heads x 64):
  Q = Xq @ (Wq/temp)^T, K = Xk @ Wk^T, V = Xv @ Wv^T
  per head: A = softmax(Q K^T); O = A V
  O = silu(O); LN (affine folded into fc); out_pre = veff + xhat @ wfcT_eff
  out = BatchNorm2d(out_pre) with global batch stats (AllReduce over 8 cores)

v2 structure (vs baseline):
  * Stream order: v(bf16) -> qk(fp8) -> veff -> wfc, 6KB DMA lines (4 p-chunks
    per transfer). V completes at ~1/3 of the stream, so attention, LN and fc
    all overlap the remaining transfers; fc is paced by wfc arrival.
  * Q,K projections in fp8 (e4m3) DoubleRow, weights-stationary: Q^T/K^T land
    in PSUM directly (no PE transposes). Scales: acts x16, wq/temp x2^19,
    wk x2^13; descale folded into the exp() activation scale.
  * V projection bf16 activation-stationary -> V[c,d] direct.
  * BN channel sums via matmul against host-precomputed wfc column sums +
    host veff row-sums; only sum(x^2) is accumulated on-chip (Square accum).
  * Scalar activations stay on one table where possible: sigmoid computed as
    1/(1+exp(-x)) reusing the Exp table; table switch only for Sqrt/Square.

Sharding: data-parallel over batch, 2 per core, 8 cores; [128,2] AllReduce
for BN stats (+ warm-up AllReduce at start).
"""

import os

import numpy as np

import concourse.mybir as mybir
import concourse.tile as tile
from concourse import bacc
from concourse.bass_utils import run_bass_kernel_spmd
from concourse.masks import make_identity

# ---- problem constants (hardcoded per contract) ----
B, C, HH, WW = 16, 128, 64, 64
P = HH * WW           # 4096
NH, LD = 8, 64
D = NH * LD           # 512
N_CORES = 8
BPC = B // N_CORES    # 2 batches per core
NG = 8                # DMA groups (4 p-chunks of 128 each -> 512 P per group)
NPT = 8               # 512-col output tiles over P
LN_EPS = 1e-6
BN_EPS = 1e-5
F32 = mybir.dt.float32
BF16 = mybir.dt.bfloat16
FP8 = mybir.dt.float8e4

# power-of-2 scales for fp8 quantization
SA = 2.0 ** 4      # q,k activations
SWQ = 2.0 ** 19    # wq/temp
SWK = 2.0 ** 13    # wk
EXP_SCALE = 1.0 / (SA * SA * SWQ * SWK)   # descale folded into exp()

MODE = "v2-qk8"
QK_BF16 = os.environ.get("BASS_QK_BF16", "0") == "1"
BN_LOCAL = os.environ.get("BASS_BN_LOCAL", "0") == "1"
CC_WARM = os.environ.get("BASS_CC_WARM", "1") == "1"
STOP_AFTER = os.environ.get("BASS_STOP_AFTER", "")

_BUILD_CACHE: dict = {}
LAST_RESULTS = None


def _emit(ctx, nc, tc, io):
    AF = mybir.ActivationFunctionType
    ALU = mybir.AluOpType
    AX = mybir.AxisListType
    PM = mybir.MatmulPerfMode

    consts = ctx.enter_context(tc.tile_pool(name="consts", bufs=1))
    vpool = ctx.enter_context(tc.tile_pool(name="vpool", bufs=8))
    qkpool = ctx.enter_context(tc.tile_pool(name="qkpool", bufs=8))
    fcpool = ctx.enter_context(tc.tile_pool(name="fcpool", bufs=1))
    sb = ctx.enter_context(tc.tile_pool(name="sb", bufs=2))
    keep = ctx.enter_context(tc.tile_pool(name="keep", bufs=1))
    small = ctx.enter_context(tc.tile_pool(name="small", bufs=4))
    stat = ctx.enter_context(tc.tile_pool(name="stat", bufs=1))
    dram = ctx.enter_context(tc.tile_pool(name="dram", bufs=1, space="DRAM"))

    ident = consts.tile([128, 128], BF16, tag="ident", name="ident")
    ident_f = consts.tile([128, 128], F32, tag="identf", name="identf")
    make_identity(nc, ident_f)
    nc.vector.tensor_copy(out=ident, in_=ident_f)

    bng = consts.tile([128, 1], F32, tag="bng", name="bng")
    bnb = consts.tile([128, 1], F32, tag="bnb", name="bnb")
    epsbn = consts.tile([128, 1], F32, tag="epsbn", name="epsbn")
    vsum = consts.tile([128, 1], F32, tag="vsum", name="vsum")
    colsum = consts.tile([128, 4], BF16, tag="colsum", name="colsum")
    nc.gpsimd.dma_start(out=bng, in_=io["bng"][:, :])
    nc.gpsimd.dma_start(out=bnb, in_=io["bnb"][:, :])
    nc.gpsimd.dma_start(out=vsum, in_=io["vsum"][:, :])
    nc.gpsimd.dma_start(out=colsum, in_=io["colsum"][:, :])
    nc.vector.memset(epsbn, BN_EPS)
    # preload the Exp table while the v stream runs
    extab = consts.tile([128, 1], F32, tag="extab", name="extab")
    nc.scalar.activation(out=extab, in_=epsbn, func=AF.Exp)

    # V_sb rows: per (b, h) 65 cols = [V_h | 1] so the AV matmul also emits
    # the softmax denominator
    V_sb = keep.tile([128, BPC, NH, 65], BF16, tag="V_sb", name="V_sb")
    for b in range(BPC):
        for h in range(NH):
            nc.vector.memset(V_sb[:, b, h, 64:65], 1.0)

    out_sb = [keep.tile([128, P], BF16, tag=f"osb{b}", name=f"osb{b}")
              for b in range(BPC)]
    pstack = []

    def penter(ctxmgr):
        pool = ctxmgr.__enter__()
        pstack.append(ctxmgr)
        return pool

    def pexit():
        pstack.pop().__exit__(None, None, None)

    def bail():
        while pstack:
            pexit()
        for b in range(BPC):
            nc.vector.memset(out_sb[b], 0.0)
            nc.sync.dma_start(out=io["out"][b, :, :], in_=out_sb[b])

    # ---- qk stream: W-stationary fp8 DoubleRow projections -> Q^T, K^T ----
    # qk8[g]: [128 p, 4 pc, 1536] = [q b0c|q b1c|k b0c|k b1c|wq 512d|wk 512d]
    ps_qk = penter(tc.tile_pool(name="ps_qk", bufs=1, space="PSUM"))
    QTp = [ps_qk.tile([128, 512], F32, tag=f"QTp{t}", name=f"QTp{t}") for t in range(4)]
    KTp = [ps_qk.tile([128, 512], F32, tag=f"KTp{t}", name=f"KTp{t}") for t in range(4)]
    for g in range(NG):
        qa = qkpool.tile([128, 4, 1536], BF16 if QK_BF16 else FP8,
                         tag="qa", name="qa")
        (nc.sync if g % 2 == 0 else nc.gpsimd).dma_start(out=qa, in_=io["qk8"][g])
        for dc in range(4):
            for dst, wo, ao in ((QTp, 512, 0), (KTp, 1024, 256)):
                if QK_BF16:
                    for j in range(4):
                        pc = 4 * g + j
                        nc.tensor.matmul(
                            dst[dc][:, 0:256],
                            qa[:, j, wo + dc * 128:wo + (dc + 1) * 128],
                            qa[:, j, ao:ao + 256],
                            start=pc == 0, stop=pc == 4 * NG - 1)
                else:
                    for jj in range(2):
                        pr = 2 * g + jj
                        nc.tensor.matmul(
                            dst[dc][:, 0:256],
                            qa[:, 2 * jj:2 * jj + 2, wo + dc * 128:wo + (dc + 1) * 128],
                            qa[:, 2 * jj:2 * jj + 2, ao:ao + 256],
                            start=pr == 0, stop=pr == 2 * NG - 1,
                            perf_mode=PM.DoubleRow)

    # ---- evac Q^T/K^T (vector; scalar stays on the Exp table) ----
    QT_sb = keep.tile([128, 4, 256], BF16, tag="QT_sb", name="QT_sb")
    KT_sb = keep.tile([128, 4, 256], BF16, tag="KT_sb", name="KT_sb")
    for t in range(4):
        nc.vector.tensor_copy(out=QT_sb[:, t, :], in_=QTp[t][:, 0:256])
        nc.vector.tensor_copy(out=KT_sb[:, t, :], in_=KTp[t][:, 0:256])
    pexit()
    if STOP_AFTER == "qk":
        return bail()

    # ---- S^T + exp per head (runtime rejects packed multi-group S tiles);
    # emitted before the v matmuls so they run during the v stream ----
    ps_s = penter(tc.tile_pool(name="ps_s", bufs=2, space="PSUM"))
    AT_all = keep.tile([128, BPC, NH, 128], BF16, tag="AT", name="AT")

    def qk_slice(sbuf, h, b):
        return sbuf[(h % 2) * 64:(h % 2) * 64 + 64, h // 2, b * 128:(b + 1) * 128]

    for idx in range(BPC * NH):
        b, h = divmod(idx, NH)
        S = ps_s.tile([128, 128], F32, tag="Ssm", name="Ssm")
        nc.tensor.matmul(S[:, :], qk_slice(KT_sb, h, b), qk_slice(QT_sb, h, b),
                         start=True, stop=True)
        nc.scalar.activation(out=AT_all[:, b, h, :], in_=S[:, :], func=AF.Exp,
                             scale=1.0 if QK_BF16 else EXP_SCALE)

    # ---- v stream: acts-stationary projection -> V[c, d] ----
    # vkb[g]: [128 p, 4 pc, 768] = [v b0 c | v b1 c | wv 512 d]
    ps_v = penter(tc.tile_pool(name="ps_v", bufs=1, space="PSUM"))
    Vp = [ps_v.tile([128, 512], F32, tag=f"Vp{b}", name=f"Vp{b}") for b in range(BPC)]
    for g in range(NG):
        va = vpool.tile([128, 4, 768], BF16, tag="va", name="va")
        (nc.sync if g % 2 == 0 else nc.gpsimd).dma_start(out=va, in_=io["vkb"][g])
        for j in range(4):
            pc = 4 * g + j
            for b in range(BPC):
                nc.tensor.matmul(Vp[b][:, :], va[:, j, b * 128:(b + 1) * 128],
                                 va[:, j, 256:768],
                                 start=pc == 0, stop=pc == 4 * NG - 1)

    # warm-up AllReduce: absorbs one-time CC init + launch skew; triggered
    # here so it completes before the real stats AllReduce
    if CC_WARM and not BN_LOCAL:
        dumin = dram.tile([128, 1], F32, tag="dumin", name="dumin")
        dumout = dram.tile([128, 1], F32, tag="dumout", name="dumout")
        nc.gpsimd.dma_start(out=dumin[:, :], in_=epsbn)
        nc.gpsimd.collective_compute(
            "AllReduce", ALU.add, replica_groups=[list(range(N_CORES))],
            ins=[dumin.opt()], outs=[dumout.opt()])

    # late streams: veff then wfc (fc is paced by wfc arrival)
    veffs = []
    for b in range(BPC):
        t = keep.tile([128, P], BF16, tag=f"veff{b}", name=f"veff{b}")
        (nc.sync if b % 2 == 0 else nc.gpsimd).dma_start(out=t, in_=io["veff"][b, :, :])
        veffs.append(t)
    wfcts = []
    for pt in range(NPT):
        wfct = fcpool.tile([128, 4, 512], BF16, tag=f"wfct{pt}", name=f"wfct{pt}")
        (nc.sync if pt % 2 == 0 else nc.gpsimd).dma_start(out=wfct, in_=io["wfc"][pt])
        wfcts.append(wfct)

    # V evac on vector
    for b in range(BPC):
        for h in range(NH):
            nc.vector.tensor_copy(out=V_sb[:, b, h, 0:64],
                                  in_=Vp[b][:, h * 64:(h + 1) * 64])
    pexit()
    pexit()
    if STOP_AFTER == "v":
        return bail()

    ps_b = penter(tc.tile_pool(name="ps_b", bufs=4, space="PSUM"))
    ps_od = penter(tc.tile_pool(name="ps_od", bufs=2, space="PSUM"))
    ps_t = penter(tc.tile_pool(name="ps_t", bufs=1, space="PSUM"))

    # ---- AV + normalize ----
    Oscs = []
    for b in range(BPC):
        Osc = sb.tile([128, D], F32, tag="Osc", name=f"Osc{b}")
        for h in range(NH):
            Od = ps_od.tile([128, 65], F32, tag="Od", name="Od")
            nc.tensor.matmul(Od[:, :], AT_all[:, b, h, :], V_sb[:, b, h, :],
                             start=True, stop=True)
            rs = small.tile([128, 1], F32, tag="rs", name="rs")
            nc.vector.reciprocal(rs, Od[:, 64:65])
            nc.vector.tensor_scalar_mul(out=Osc[:, h * 64:(h + 1) * 64],
                                        in0=Od[:, 0:64], scalar1=rs)
        Oscs.append(Osc)
    if STOP_AFTER == "attn":
        return bail()

    # ---- per-batch: silu (scalar Sigmoid) + LN + xhat + xT + colsum, then
    # that batch's fc immediately (b0's fc overlaps b1's LN chain) ----
    msum = ps_od.tile([128, 1], F32, tag="msum", name="msum", bufs=1)
    pcols = stat.tile([128, 16], F32, tag="pcols", name="pcols")
    junk = keep.tile([128, 512], BF16, tag="junk", name="junk")

    def ln_chain(b):
        sg = sb.tile([128, D], F32, tag="sg", name=f"sg{b}")
        nc.scalar.activation(out=sg, in_=Oscs[b], func=AF.Sigmoid)
        Osw = sb.tile([128, D], F32, tag="Osw", name=f"Osw{b}")
        nc.vector.tensor_mul(out=Osw, in0=Oscs[b], in1=sg)
        st6 = small.tile([128, 6], F32, tag="st6", name="st6")
        nc.vector.bn_stats(out=st6, in_=Osw)
        mv = small.tile([128, 2], F32, tag="mv", name=f"mv{b}")
        nc.vector.bn_aggr(out=mv, in_=st6)
        sd = small.tile([128, 1], F32, tag="sd", name="sd")
        nc.scalar.activation(out=sd, in_=mv[:, 1:2], func=AF.Sqrt,
                             scale=float(D) / (D - 1))
        nc.vector.tensor_scalar_add(out=sd, in0=sd, scalar1=LN_EPS)
        rstd = small.tile([128, 1], F32, tag="rstd", name=f"rstd{b}")
        nc.vector.reciprocal(rstd, sd)
        xhat = sb.tile([128, D], BF16, tag="xhat", name=f"xhat{b}")
        nc.vector.tensor_scalar(out=xhat, in0=Osw, scalar1=mv[:, 0:1],
                                scalar2=rstd, op0=ALU.subtract, op1=ALU.mult)
        xT = sb.tile([128, D], BF16, tag="xT", name=f"xT{b}")
        for dc in range(4):
            tp = ps_t.tile([128, 128], BF16, tag="stp", name="stp")
            nc.tensor.transpose(tp[:, :], xhat[:, dc * 128:(dc + 1) * 128],
                                ident[:, :])
            nc.vector.tensor_copy(out=xT[:, dc * 128:(dc + 1) * 128], in_=tp[:, :])
        for dc in range(4):
            nc.tensor.matmul(msum[:, :], xT[:, dc * 128:(dc + 1) * 128],
                             colsum[:, dc:dc + 1],
                             start=(b == 0 and dc == 0), stop=(b == 1 and dc == 3))
        return xT

    def fc_batch(b, xT):
        # dc-outer over 4-pt halves: one LDWEIGHTS per 4 N=512 matmuls
        for half in range(2):
            O2s = [ps_b.tile([128, 512], F32, tag="O2", name=f"O2{pt}")
                   for pt in range(half * 4, half * 4 + 4)]
            for dc in range(4):
                for k, pt in enumerate(range(half * 4, half * 4 + 4)):
                    nc.tensor.matmul(O2s[k][:, :],
                                     xT[:, dc * 128:(dc + 1) * 128],
                                     wfcts[pt][:, dc, :],
                                     start=dc == 0, stop=dc == 3)
            for k, pt in enumerate(range(half * 4, half * 4 + 4)):
                seg = out_sb[b][:, pt * 512:(pt + 1) * 512]
                nc.vector.tensor_add(out=seg, in0=O2s[k][:, :],
                                     in1=veffs[b][:, pt * 512:(pt + 1) * 512])
                nc.scalar.activation(out=junk, in_=seg, func=AF.Square,
                                     accum_out=pcols[:, pt * 2 + b:pt * 2 + b + 1])

    xT0 = ln_chain(0)
    xT1 = ln_chain(1)
    fc_batch(0, xT0)
    fc_batch(1, xT1)

    stats2 = stat.tile([128, 2], F32, tag="stats2", name="stats2")
    nc.vector.tensor_add(out=stats2[:, 0:1], in0=msum[:, :], in1=vsum)
    nc.vector.reduce_sum(stats2[:, 1:2], pcols, axis=AX.X)
    if STOP_AFTER == "fc":
        return bail()

    # ---- BN stats AllReduce ----
    if BN_LOCAL:
        red = stats2
        inv_n = 1.0 / float(BPC * P)
    else:
        cin = dram.tile([128, 2], F32, tag="cin", name="cin")
        cout = dram.tile([128, 2], F32, tag="cout", name="cout")
        nc.gpsimd.dma_start(out=cin[:, :], in_=stats2)
        nc.gpsimd.collective_compute(
            "AllReduce", ALU.add, replica_groups=[list(range(N_CORES))],
            ins=[cin.opt()], outs=[cout.opt()])
        red = stat.tile([128, 2], F32, tag="red", name="red")
        nc.gpsimd.dma_start(out=red[:, :], in_=cout[:, :])
        inv_n = 1.0 / float(B * P)

    # ---- BN math (vector-heavy; one scalar Sqrt) ----
    t2 = small.tile([128, 2], F32, tag="t2", name="t2")
    nc.vector.tensor_scalar_mul(out=t2, in0=red, scalar1=inv_n)
    msq = small.tile([128, 1], F32, tag="msq", name="msq")
    nc.vector.tensor_mul(out=msq, in0=t2[:, 0:1], in1=t2[:, 0:1])
    var = small.tile([128, 1], F32, tag="var", name="var")
    nc.vector.tensor_sub(out=var, in0=t2[:, 1:2], in1=msq)
    sdv = small.tile([128, 1], F32, tag="sdv", name="sdv")
    nc.scalar.activation(out=sdv, in_=var, func=AF.Sqrt, bias=epsbn)
    invs = small.tile([128, 1], F32, tag="invs", name="invs")
    nc.vector.reciprocal(invs, sdv)
    scl = small.tile([128, 1], F32, tag="scl", name="scl")
    nc.vector.tensor_mul(out=scl, in0=bng, in1=invs)
    tmp = small.tile([128, 1], F32, tag="tmp", name="tmp")
    nc.vector.tensor_mul(out=tmp, in0=t2[:, 0:1], in1=scl)
    shf = small.tile([128, 1], F32, tag="shf", name="shf")
    nc.vector.tensor_sub(out=shf, in0=bnb, in1=tmp)

    # ---- apply + store (8 chunks, stores alternate sync/gpsimd queues) ----
    store_q = [nc.sync, nc.gpsimd, nc.scalar, nc.sync]
    for b in range(BPC):
        for hf in range(2):
            seg = out_sb[b][:, hf * 2048:(hf + 1) * 2048]
            nc.vector.tensor_scalar(out=seg, in0=seg, scalar1=scl, scalar2=shf,
                                    op0=ALU.mult, op1=ALU.add)
            eng = store_q[(b * 2 + hf) % 4]
            eng.dma_start(out=io["out"][b, :, hf * 2048:(hf + 1) * 2048], in_=seg)

    while pstack:
        pexit()


def _build():
    key = (MODE, QK_BF16, BN_LOCAL, CC_WARM, STOP_AFTER)
    if key in _BUILD_CACHE:
        return _BUILD_CACHE[key]
    nc = bacc.Bacc("TRN2", target_bir_lowering=False, debug=False,
                   num_devices=N_CORES)
    io = {
        "qk8": nc.dram_tensor("qk8", [NG, 128, 4, 1536],
                              BF16 if QK_BF16 else FP8,
                              kind="ExternalInput").ap(),
        "vkb": nc.dram_tensor("vkb", [NG, 128, 4, 768], BF16,
                              kind="ExternalInput").ap(),
        "veff": nc.dram_tensor("veff", [BPC, C, P], BF16, kind="ExternalInput").ap(),
        "wfc": nc.dram_tensor("wfc", [NPT, 128, 4, 512], BF16,
                              kind="ExternalInput").ap(),
        "colsum": nc.dram_tensor("colsum", [128, 4], BF16, kind="ExternalInput").ap(),
        "vsum": nc.dram_tensor("vsum", [C, 1], F32, kind="ExternalInput").ap(),
        "bng": nc.dram_tensor("bng", [C, 1], F32, kind="ExternalInput").ap(),
        "bnb": nc.dram_tensor("bnb", [C, 1], F32, kind="ExternalInput").ap(),
        "out": nc.dram_tensor("out", [BPC, C, P], BF16, kind="ExternalOutput").ap(),
    }
    from contextlib import ExitStack
    with tile.TileContext(nc) as tc, ExitStack() as ctx:
        _emit(ctx, nc, tc, io)
    nc.compile()
    _BUILD_CACHE[key] = nc
    return nc


def _bf16(x):
    import ml_dtypes
    return np.ascontiguousarray(np.asarray(x, np.float32).astype(ml_dtypes.bfloat16))


def _fp8(x, s):
    import ml_dtypes
    y = np.clip(np.asarray(x, np.float32) * s, -240.0, 240.0)
    return np.ascontiguousarray(y.astype(ml_dtypes.float8_e4m3))


def kernel(v, k, q, w_qs, w_ks, w_vs, w_fc, ln_gamma, ln_beta, temperature,
           bn_gamma, bn_beta, **_ignored):
    v = np.asarray(v, np.float32)
    k = np.asarray(k, np.float32)
    q = np.asarray(q, np.float32)
    w_qs = np.asarray(w_qs, np.float32)
    w_ks = np.asarray(w_ks, np.float32)
    w_vs = np.asarray(w_vs, np.float32)
    w_fc = np.asarray(w_fc, np.float32)
    ln_gamma = np.asarray(ln_gamma, np.float32)
    ln_beta = np.asarray(ln_beta, np.float32)
    temp = float(np.asarray(temperature))
    bn_gamma = np.asarray(bn_gamma, np.float32)
    bn_beta = np.asarray(bn_beta, np.float32)

    qf = q.reshape(B, C, P)
    kf = k.reshape(B, C, P)
    vf = v.reshape(B, C, P)

    def tr_chunks(X):
        # [B, C, P] -> [core, g, p, j, b*128+c]
        Y = X.reshape(N_CORES, BPC, C, NG, 4, 128)       # core b c g j p
        return Y.transpose(0, 3, 5, 4, 1, 2).reshape(N_CORES, NG, 128, 4, 256)

    def wT_chunks(W):
        # [D, P] -> [g, p, j, d]
        return W.reshape(D, NG, 4, 128).transpose(1, 3, 2, 0)

    # qk8 pack: [core, g, p, j, q 256 | k 256 | wq 512 | wk 512]
    if QK_BF16:
        qa_q, qa_k = _bf16(tr_chunks(qf)), _bf16(tr_chunks(kf))
        wq_c = _bf16(wT_chunks(w_qs / temp))
        wk_c = _bf16(wT_chunks(w_ks))
    else:
        qa_q, qa_k = _fp8(tr_chunks(qf), SA), _fp8(tr_chunks(kf), SA)
        wq_c = _fp8(wT_chunks(w_qs / temp), SWQ)
        wk_c = _fp8(wT_chunks(w_ks), SWK)
    qk8 = np.ascontiguousarray(np.concatenate([
        qa_q, qa_k,
        np.broadcast_to(wq_c[None], (N_CORES,) + wq_c.shape),
        np.broadcast_to(wk_c[None], (N_CORES,) + wk_c.shape)], axis=-1))

    # vkb pack: [core, g, p, j, v 256 | wv 512]
    wv_c = _bf16(wT_chunks(w_vs))
    vkb = np.ascontiguousarray(np.concatenate(
        [_bf16(tr_chunks(vf)),
         np.broadcast_to(wv_c[None], (N_CORES,) + wv_c.shape)], axis=-1))

    # wfc pack [pt, d-sub, dc, p-col]: wfcT_eff[dc*128+d, pt*512+p]
    wfcT_eff = _bf16((w_fc * ln_gamma[None, :]).T)     # [D, P] bf16
    wfc = np.ascontiguousarray(
        wfcT_eff.reshape(4, 128, NPT, 512).transpose(2, 1, 0, 3))
    colsum64 = np.asarray(wfcT_eff, np.float64).sum(axis=1)   # [D]
    colsum = _bf16(colsum64.reshape(4, 128).T)                # [128, 4]

    bias_fc = (w_fc @ ln_beta).astype(np.float32)
    veff = _bf16(vf + bias_fc[None, None, :])                 # [B, C, P] bf16
    vsum_all = np.asarray(veff, np.float64).sum(axis=2)       # [B, C]
    bng = np.ascontiguousarray(bn_gamma.reshape(C, 1))
    bnb = np.ascontiguousarray(bn_beta.reshape(C, 1))

    nc = _build()
    in_maps = []
    for i in range(N_CORES):
        bs = slice(BPC * i, BPC * (i + 1))
        in_maps.append({
            "qk8": qk8[i], "vkb": vkb[i], "veff": veff[bs], "wfc": wfc,
            "colsum": colsum,
            "vsum": np.ascontiguousarray(
                vsum_all[bs].sum(axis=0).astype(np.float32).reshape(C, 1)),
            "bng": bng, "bnb": bnb,
        })
    res = run_bass_kernel_spmd(nc, in_maps, core_ids=list(range(N_CORES)))
    global LAST_RESULTS
    LAST_RESULTS = res
    out = np.concatenate([np.asarray(res.results[i]["out"])
                          for i in range(N_CORES)], axis=0)
    return out.reshape(B, C, HH, WW).astype(np.float32)


# revision 3
# speedup vs baseline: 1.1605x; 1.1605x over previous
"""Trainium2 Bass kernel v2 for nn_MultiHeadAttention (channel-attention block).

Math per batch (X* = reshape(*, [C,P]), P=4096, C=128, D=512, 8 heads x 64):
  Q = Xq @ (Wq/temp)^T, K = Xk @ Wk^T, V = Xv @ Wv^T
  per head: A = softmax(Q K^T); O = A V
  O = silu(O); LN (affine folded into fc); out_pre = veff + xhat @ wfcT_eff
  out = BatchNorm2d(out_pre) with global batch stats (AllReduce over 8 cores)

v2 structure (vs baseline):
  * Stream order: v(bf16) -> qk(fp8) -> veff -> wfc, 6KB DMA lines (4 p-chunks
    per transfer). V completes at ~1/3 of the stream, so attention, LN and fc
    all overlap the remaining transfers; fc is paced by wfc arrival.
  * Q,K projections in fp8 (e4m3) DoubleRow, weights-stationary: Q^T/K^T land
    in PSUM directly (no PE transposes). Scales: acts x16, wq/temp x2^19,
    wk x2^13; descale folded into the exp() activation scale.
  * V projection bf16 activation-stationary -> V[c,d] direct.
  * BN channel sums via matmul against host-precomputed wfc column sums +
    host veff row-sums; only sum(x^2) is accumulated on-chip (Square accum).
  * Scalar activations stay on one table where possible: sigmoid computed as
    1/(1+exp(-x)) reusing the Exp table; table switch only for Sqrt/Square.

Sharding: data-parallel over batch, 2 per core, 8 cores; [128,2] AllReduce
for BN stats (+ warm-up AllReduce at start).
"""

import os

import numpy as np

import concourse.mybir as mybir
import concourse.tile as tile
from concourse import bacc
from concourse.bass_utils import run_bass_kernel_spmd
from concourse.masks import make_identity

# ---- problem constants (hardcoded per contract) ----
B, C, HH, WW = 16, 128, 64, 64
P = HH * WW           # 4096
NH, LD = 8, 64
D = NH * LD           # 512
N_CORES = 8
BPC = B // N_CORES    # 2 batches per core
NG = 8                # DMA groups (4 p-chunks of 128 each -> 512 P per group)
NPT = 8               # 512-col output tiles over P
LN_EPS = 1e-6
BN_EPS = 1e-5
F32 = mybir.dt.float32
BF16 = mybir.dt.bfloat16
FP8 = mybir.dt.float8e4

# power-of-2 scales for fp8 quantization
SA = 2.0 ** 4      # q,k activations
SWQ = 2.0 ** 19    # wq/temp
SWK = 2.0 ** 13    # wk
SWV = 2.0 ** 13    # wv
EXP_SCALE = 1.0 / (SA * SA * SWQ * SWK)   # descale folded into exp()

MODE = "v2-qk8"
QK_BF16 = os.environ.get("BASS_QK_BF16", "0") == "1"
BN_LOCAL = os.environ.get("BASS_BN_LOCAL", "0") == "1"
CC_WARM = os.environ.get("BASS_CC_WARM", "1") == "1"
STOP_AFTER = os.environ.get("BASS_STOP_AFTER", "")

_BUILD_CACHE: dict = {}
LAST_RESULTS = None


def _emit(ctx, nc, tc, io):
    AF = mybir.ActivationFunctionType
    ALU = mybir.AluOpType
    AX = mybir.AxisListType
    PM = mybir.MatmulPerfMode

    consts = ctx.enter_context(tc.tile_pool(name="consts", bufs=1))
    vpool = ctx.enter_context(tc.tile_pool(name="vpool", bufs=8))
    qkpool = ctx.enter_context(tc.tile_pool(name="qkpool", bufs=8))
    fcpool = ctx.enter_context(tc.tile_pool(name="fcpool", bufs=1))
    sb = ctx.enter_context(tc.tile_pool(name="sb", bufs=2))
    keep = ctx.enter_context(tc.tile_pool(name="keep", bufs=1))
    small = ctx.enter_context(tc.tile_pool(name="small", bufs=4))
    stat = ctx.enter_context(tc.tile_pool(name="stat", bufs=1))
    dram = ctx.enter_context(tc.tile_pool(name="dram", bufs=1, space="DRAM"))

    ident = consts.tile([128, 128], BF16, tag="ident", name="ident")
    ident_f = consts.tile([128, 128], F32, tag="identf", name="identf")
    make_identity(nc, ident_f)
    nc.vector.tensor_copy(out=ident, in_=ident_f)

    bng = consts.tile([128, 1], F32, tag="bng", name="bng")
    bnb = consts.tile([128, 1], F32, tag="bnb", name="bnb")
    epsbn = consts.tile([128, 1], F32, tag="epsbn", name="epsbn")
    vsum = consts.tile([128, 1], F32, tag="vsum", name="vsum")
    colsum = consts.tile([128, 4], BF16, tag="colsum", name="colsum")
    nc.gpsimd.dma_start(out=bng, in_=io["bng"][:, :])
    nc.gpsimd.dma_start(out=bnb, in_=io["bnb"][:, :])
    nc.gpsimd.dma_start(out=vsum, in_=io["vsum"][:, :])
    nc.gpsimd.dma_start(out=colsum, in_=io["colsum"][:, :])
    nc.vector.memset(epsbn, BN_EPS)
    # preload the Exp table while the v stream runs
    extab = consts.tile([128, 1], F32, tag="extab", name="extab")
    nc.scalar.activation(out=extab, in_=epsbn, func=AF.Exp)

    # V_sb rows: per (b, h) 65 cols = [V_h | 1] so the AV matmul also emits
    # the softmax denominator
    V_sb = keep.tile([128, BPC, NH, 65], BF16, tag="V_sb", name="V_sb")
    for b in range(BPC):
        for h in range(NH):
            nc.vector.memset(V_sb[:, b, h, 64:65], 1.0)

    out_sb = [keep.tile([128, P], BF16, tag=f"osb{b}", name=f"osb{b}")
              for b in range(BPC)]
    pstack = []

    def penter(ctxmgr):
        pool = ctxmgr.__enter__()
        pstack.append(ctxmgr)
        return pool

    def pexit():
        pstack.pop().__exit__(None, None, None)

    def bail():
        while pstack:
            pexit()
        for b in range(BPC):
            nc.vector.memset(out_sb[b], 0.0)
            nc.sync.dma_start(out=io["out"][b, :, :], in_=out_sb[b])

    # ---- qk stream: W-stationary fp8 DoubleRow projections -> Q^T, K^T ----
    # qk8[g]: [128 p, 4 pc, 1536] = [q b0c|q b1c|k b0c|k b1c|wq 512d|wk 512d]
    ps_qk = penter(tc.tile_pool(name="ps_qk", bufs=1, space="PSUM"))
    QTp = [ps_qk.tile([128, 512], F32, tag=f"QTp{t}", name=f"QTp{t}") for t in range(4)]
    KTp = [ps_qk.tile([128, 512], F32, tag=f"KTp{t}", name=f"KTp{t}") for t in range(4)]
    for g in range(NG):
        qa = qkpool.tile([128, 4, 1536], BF16 if QK_BF16 else FP8,
                         tag="qa", name="qa")
        (nc.sync if g % 2 == 0 else nc.gpsimd).dma_start(out=qa, in_=io["qk8"][g])
        for dc in range(4):
            for dst, wo, ao in ((QTp, 512, 0), (KTp, 1024, 256)):
                if QK_BF16:
                    for j in range(4):
                        pc = 4 * g + j
                        nc.tensor.matmul(
                            dst[dc][:, 0:256],
                            qa[:, j, wo + dc * 128:wo + (dc + 1) * 128],
                            qa[:, j, ao:ao + 256],
                            start=pc == 0, stop=pc == 4 * NG - 1)
                else:
                    for jj in range(2):
                        pr = 2 * g + jj
                        nc.tensor.matmul(
                            dst[dc][:, 0:256],
                            qa[:, 2 * jj:2 * jj + 2, wo + dc * 128:wo + (dc + 1) * 128],
                            qa[:, 2 * jj:2 * jj + 2, ao:ao + 256],
                            start=pr == 0, stop=pr == 2 * NG - 1,
                            perf_mode=PM.DoubleRow)

    # ---- evac Q^T/K^T (vector; scalar stays on the Exp table) ----
    QT_sb = keep.tile([128, 4, 256], BF16, tag="QT_sb", name="QT_sb")
    KT_sb = keep.tile([128, 4, 256], BF16, tag="KT_sb", name="KT_sb")
    for t in range(4):
        nc.vector.tensor_copy(out=QT_sb[:, t, :], in_=QTp[t][:, 0:256])
        nc.vector.tensor_copy(out=KT_sb[:, t, :], in_=KTp[t][:, 0:256])
    pexit()
    if STOP_AFTER == "qk":
        return bail()

    # ---- S^T + exp per head (runtime rejects packed multi-group S tiles);
    # emitted before the v matmuls so they run during the v stream ----
    ps_s = penter(tc.tile_pool(name="ps_s", bufs=2, space="PSUM"))
    AT_all = keep.tile([128, BPC, NH, 128], BF16, tag="AT", name="AT")

    def qk_slice(sbuf, h, b):
        return sbuf[(h % 2) * 64:(h % 2) * 64 + 64, h // 2, b * 128:(b + 1) * 128]

    for idx in range(BPC * NH):
        b, h = divmod(idx, NH)
        S = ps_s.tile([128, 128], F32, tag="Ssm", name="Ssm")
        nc.tensor.matmul(S[:, :], qk_slice(KT_sb, h, b), qk_slice(QT_sb, h, b),
                         start=True, stop=True)
        nc.scalar.activation(out=AT_all[:, b, h, :], in_=S[:, :], func=AF.Exp,
                             scale=1.0 if QK_BF16 else EXP_SCALE)

    # ---- v stream: acts-stationary projection -> V[c, d]; wv is resident
    # fp8 (x2^13), acts bf16 (mixed-dtype matmul), descale at evac ----
    wv8_sb = keep.tile([128, 32, 512], FP8, tag="wv8", name="wv8")
    nc.gpsimd.dma_start(out=wv8_sb, in_=io["wv8"][:, :, :])
    ps_v = penter(tc.tile_pool(name="ps_v", bufs=1, space="PSUM"))
    Vp = [ps_v.tile([128, 512], F32, tag=f"Vp{b}", name=f"Vp{b}") for b in range(BPC)]
    for g in range(NG):
        va = vpool.tile([128, 4, 256], BF16, tag="va", name="va")
        (nc.sync if g % 2 == 0 else nc.gpsimd).dma_start(out=va, in_=io["vkb"][g])
        for j in range(4):
            pc = 4 * g + j
            for b in range(BPC):
                nc.tensor.matmul(Vp[b][:, :], va[:, j, b * 128:(b + 1) * 128],
                                 wv8_sb[:, pc, :],
                                 start=pc == 0, stop=pc == 4 * NG - 1)

    # warm-up AllReduce: absorbs one-time CC init + launch skew; triggered
    # here so it completes before the real stats AllReduce
    if CC_WARM and not BN_LOCAL:
        dumin = dram.tile([128, 1], F32, tag="dumin", name="dumin")
        dumout = dram.tile([128, 1], F32, tag="dumout", name="dumout")
        nc.gpsimd.dma_start(out=dumin[:, :], in_=epsbn)
        nc.gpsimd.collective_compute(
            "AllReduce", ALU.add, replica_groups=[list(range(N_CORES))],
            ins=[dumin.opt()], outs=[dumout.opt()])

    # late streams: veff then wfc (fc is paced by wfc arrival)
    veffs = []
    for b in range(BPC):
        t = keep.tile([128, P], BF16, tag=f"veff{b}", name=f"veff{b}")
        (nc.sync if b % 2 == 0 else nc.gpsimd).dma_start(out=t, in_=io["veff"][b, :, :])
        veffs.append(t)
    wfcts = []
    for pt in range(NPT):
        wfct = fcpool.tile([128, 4, 512], BF16, tag=f"wfct{pt}", name=f"wfct{pt}")
        (nc.sync if pt % 2 == 0 else nc.gpsimd).dma_start(out=wfct, in_=io["wfc"][pt])
        wfcts.append(wfct)

    # V evac on vector (descale wv's 2^13)
    for b in range(BPC):
        for h in range(NH):
            nc.vector.tensor_scalar_mul(out=V_sb[:, b, h, 0:64],
                                        in0=Vp[b][:, h * 64:(h + 1) * 64],
                                        scalar1=1.0 / SWV)
    pexit()
    pexit()
    if STOP_AFTER == "v":
        return bail()

    ps_b = penter(tc.tile_pool(name="ps_b", bufs=4, space="PSUM"))
    ps_od = penter(tc.tile_pool(name="ps_od", bufs=2, space="PSUM"))
    ps_t = penter(tc.tile_pool(name="ps_t", bufs=1, space="PSUM"))

    # ---- AV + normalize ----
    Oscs = []
    for b in range(BPC):
        Osc = sb.tile([128, D], F32, tag="Osc", name=f"Osc{b}")
        for h in range(NH):
            Od = ps_od.tile([128, 65], F32, tag="Od", name="Od")
            nc.tensor.matmul(Od[:, :], AT_all[:, b, h, :], V_sb[:, b, h, :],
                             start=True, stop=True)
            rs = small.tile([128, 1], F32, tag="rs", name="rs")
            nc.vector.reciprocal(rs, Od[:, 64:65])
            nc.vector.tensor_scalar_mul(out=Osc[:, h * 64:(h + 1) * 64],
                                        in0=Od[:, 0:64], scalar1=rs)
        Oscs.append(Osc)
    if STOP_AFTER == "attn":
        return bail()

    # ---- per-batch: silu (scalar Sigmoid) + LN + xhat + xT + colsum, then
    # that batch's fc immediately (b0's fc overlaps b1's LN chain) ----
    msum = ps_od.tile([128, 1], F32, tag="msum", name="msum", bufs=1)
    pcols = stat.tile([128, 16], F32, tag="pcols", name="pcols")
    junk = keep.tile([128, 512], BF16, tag="junk", name="junk")

    def ln_chain(b):
        sg = sb.tile([128, D], F32, tag="sg", name=f"sg{b}")
        nc.scalar.activation(out=sg, in_=Oscs[b], func=AF.Sigmoid)
        Osw = sb.tile([128, D], F32, tag="Osw", name=f"Osw{b}")
        nc.vector.tensor_mul(out=Osw, in0=Oscs[b], in1=sg)
        st6 = small.tile([128, 6], F32, tag="st6", name="st6")
        nc.vector.bn_stats(out=st6, in_=Osw)
        mv = small.tile([128, 2], F32, tag="mv", name=f"mv{b}")
        nc.vector.bn_aggr(out=mv, in_=st6)
        sd = small.tile([128, 1], F32, tag="sd", name="sd")
        nc.scalar.activation(out=sd, in_=mv[:, 1:2], func=AF.Sqrt,
                             scale=float(D) / (D - 1))
        nc.vector.tensor_scalar_add(out=sd, in0=sd, scalar1=LN_EPS)
        rstd = small.tile([128, 1], F32, tag="rstd", name=f"rstd{b}")
        nc.vector.reciprocal(rstd, sd)
        xhat = sb.tile([128, D], BF16, tag="xhat", name=f"xhat{b}")
        nc.vector.tensor_scalar(out=xhat, in0=Osw, scalar1=mv[:, 0:1],
                                scalar2=rstd, op0=ALU.subtract, op1=ALU.mult)
        xT = sb.tile([128, D], BF16, tag="xT", name=f"xT{b}")
        for dc in range(4):
            tp = ps_t.tile([128, 128], BF16, tag="stp", name="stp")
            nc.tensor.transpose(tp[:, :], xhat[:, dc * 128:(dc + 1) * 128],
                                ident[:, :])
            nc.vector.tensor_copy(out=xT[:, dc * 128:(dc + 1) * 128], in_=tp[:, :])
        for dc in range(4):
            nc.tensor.matmul(msum[:, :], xT[:, dc * 128:(dc + 1) * 128],
                             colsum[:, dc:dc + 1],
                             start=(b == 0 and dc == 0), stop=(b == 1 and dc == 3))
        return xT

    def fc_batch(b, xT):
        # dc-outer over 4-pt halves: one LDWEIGHTS per 4 N=512 matmuls
        for half in range(2):
            O2s = [ps_b.tile([128, 512], F32, tag="O2", name=f"O2{pt}")
                   for pt in range(half * 4, half * 4 + 4)]
            for dc in range(4):
                for k, pt in enumerate(range(half * 4, half * 4 + 4)):
                    nc.tensor.matmul(O2s[k][:, :],
                                     xT[:, dc * 128:(dc + 1) * 128],
                                     wfcts[pt][:, dc, :],
                                     start=dc == 0, stop=dc == 3)
            for k, pt in enumerate(range(half * 4, half * 4 + 4)):
                seg = out_sb[b][:, pt * 512:(pt + 1) * 512]
                nc.vector.tensor_add(out=seg, in0=O2s[k][:, :],
                                     in1=veffs[b][:, pt * 512:(pt + 1) * 512])
                nc.scalar.activation(out=junk, in_=seg, func=AF.Square,
                                     accum_out=pcols[:, pt * 2 + b:pt * 2 + b + 1])

    xT0 = ln_chain(0)
    xT1 = ln_chain(1)
    fc_batch(0, xT0)
    fc_batch(1, xT1)

    stats2 = stat.tile([128, 2], F32, tag="stats2", name="stats2")
    nc.vector.tensor_add(out=stats2[:, 0:1], in0=msum[:, :], in1=vsum)
    nc.vector.reduce_sum(stats2[:, 1:2], pcols, axis=AX.X)
    if STOP_AFTER == "fc":
        return bail()

    # ---- BN stats AllReduce ----
    if BN_LOCAL:
        red = stats2
        inv_n = 1.0 / float(BPC * P)
    else:
        cin = dram.tile([128, 2], F32, tag="cin", name="cin")
        cout = dram.tile([128, 2], F32, tag="cout", name="cout")
        nc.gpsimd.dma_start(out=cin[:, :], in_=stats2)
        nc.gpsimd.collective_compute(
            "AllReduce", ALU.add, replica_groups=[list(range(N_CORES))],
            ins=[cin.opt()], outs=[cout.opt()])
        red = stat.tile([128, 2], F32, tag="red", name="red")
        nc.gpsimd.dma_start(out=red[:, :], in_=cout[:, :])
        inv_n = 1.0 / float(B * P)

    # ---- BN math (vector-heavy; one scalar Sqrt) ----
    t2 = small.tile([128, 2], F32, tag="t2", name="t2")
    nc.vector.tensor_scalar_mul(out=t2, in0=red, scalar1=inv_n)
    msq = small.tile([128, 1], F32, tag="msq", name="msq")
    nc.vector.tensor_mul(out=msq, in0=t2[:, 0:1], in1=t2[:, 0:1])
    var = small.tile([128, 1], F32, tag="var", name="var")
    nc.vector.tensor_sub(out=var, in0=t2[:, 1:2], in1=msq)
    sdv = small.tile([128, 1], F32, tag="sdv", name="sdv")
    nc.scalar.activation(out=sdv, in_=var, func=AF.Sqrt, bias=epsbn)
    invs = small.tile([128, 1], F32, tag="invs", name="invs")
    nc.vector.reciprocal(invs, sdv)
    scl = small.tile([128, 1], F32, tag="scl", name="scl")
    nc.vector.tensor_mul(out=scl, in0=bng, in1=invs)
    tmp = small.tile([128, 1], F32, tag="tmp", name="tmp")
    nc.vector.tensor_mul(out=tmp, in0=t2[:, 0:1], in1=scl)
    shf = small.tile([128, 1], F32, tag="shf", name="shf")
    nc.vector.tensor_sub(out=shf, in0=bnb, in1=tmp)

    # ---- apply + store (8 chunks, stores alternate sync/gpsimd queues) ----
    store_q = [nc.sync, nc.gpsimd, nc.scalar, nc.sync]
    for b in range(BPC):
        for hf in range(2):
            seg = out_sb[b][:, hf * 2048:(hf + 1) * 2048]
            nc.vector.tensor_scalar(out=seg, in0=seg, scalar1=scl, scalar2=shf,
                                    op0=ALU.mult, op1=ALU.add)
            eng = store_q[(b * 2 + hf) % 4]
            eng.dma_start(out=io["out"][b, :, hf * 2048:(hf + 1) * 2048], in_=seg)

    while pstack:
        pexit()


def _build():
    key = (MODE, QK_BF16, BN_LOCAL, CC_WARM, STOP_AFTER)
    if key in _BUILD_CACHE:
        return _BUILD_CACHE[key]
    nc = bacc.Bacc("TRN2", target_bir_lowering=False, debug=False,
                   num_devices=N_CORES)
    io = {
        "qk8": nc.dram_tensor("qk8", [NG, 128, 4, 1536],
                              BF16 if QK_BF16 else FP8,
                              kind="ExternalInput").ap(),
        "vkb": nc.dram_tensor("vkb", [NG, 128, 4, 256], BF16,
                              kind="ExternalInput").ap(),
        "wv8": nc.dram_tensor("wv8", [128, 32, 512], FP8,
                              kind="ExternalInput").ap(),
        "veff": nc.dram_tensor("veff", [BPC, C, P], BF16, kind="ExternalInput").ap(),
        "wfc": nc.dram_tensor("wfc", [NPT, 128, 4, 512], BF16,
                              kind="ExternalInput").ap(),
        "colsum": nc.dram_tensor("colsum", [128, 4], BF16, kind="ExternalInput").ap(),
        "vsum": nc.dram_tensor("vsum", [C, 1], F32, kind="ExternalInput").ap(),
        "bng": nc.dram_tensor("bng", [C, 1], F32, kind="ExternalInput").ap(),
        "bnb": nc.dram_tensor("bnb", [C, 1], F32, kind="ExternalInput").ap(),
        "out": nc.dram_tensor("out", [BPC, C, P], BF16, kind="ExternalOutput").ap(),
    }
    from contextlib import ExitStack
    with tile.TileContext(nc) as tc, ExitStack() as ctx:
        _emit(ctx, nc, tc, io)
    nc.compile()
    _BUILD_CACHE[key] = nc
    return nc


def _bf16(x):
    import ml_dtypes
    return np.ascontiguousarray(np.asarray(x, np.float32).astype(ml_dtypes.bfloat16))


def _fp8(x, s):
    import ml_dtypes
    y = np.clip(np.asarray(x, np.float32) * s, -240.0, 240.0)
    return np.ascontiguousarray(y.astype(ml_dtypes.float8_e4m3))


def kernel(v, k, q, w_qs, w_ks, w_vs, w_fc, ln_gamma, ln_beta, temperature,
           bn_gamma, bn_beta, **_ignored):
    v = np.asarray(v, np.float32)
    k = np.asarray(k, np.float32)
    q = np.asarray(q, np.float32)
    w_qs = np.asarray(w_qs, np.float32)
    w_ks = np.asarray(w_ks, np.float32)
    w_vs = np.asarray(w_vs, np.float32)
    w_fc = np.asarray(w_fc, np.float32)
    ln_gamma = np.asarray(ln_gamma, np.float32)
    ln_beta = np.asarray(ln_beta, np.float32)
    temp = float(np.asarray(temperature))
    bn_gamma = np.asarray(bn_gamma, np.float32)
    bn_beta = np.asarray(bn_beta, np.float32)

    qf = q.reshape(B, C, P)
    kf = k.reshape(B, C, P)
    vf = v.reshape(B, C, P)

    def tr_chunks(X):
        # [B, C, P] -> [core, g, p, j, b*128+c]
        Y = X.reshape(N_CORES, BPC, C, NG, 4, 128)       # core b c g j p
        return Y.transpose(0, 3, 5, 4, 1, 2).reshape(N_CORES, NG, 128, 4, 256)

    def wT_chunks(W):
        # [D, P] -> [g, p, j, d]
        return W.reshape(D, NG, 4, 128).transpose(1, 3, 2, 0)

    # qk8 pack: [core, g, p, j, q 256 | k 256 | wq 512 | wk 512]
    if QK_BF16:
        qa_q, qa_k = _bf16(tr_chunks(qf)), _bf16(tr_chunks(kf))
        wq_c = _bf16(wT_chunks(w_qs / temp))
        wk_c = _bf16(wT_chunks(w_ks))
    else:
        qa_q, qa_k = _fp8(tr_chunks(qf), SA), _fp8(tr_chunks(kf), SA)
        wq_c = _fp8(wT_chunks(w_qs / temp), SWQ)
        wk_c = _fp8(wT_chunks(w_ks), SWK)
    qk8 = np.ascontiguousarray(np.concatenate([
        qa_q, qa_k,
        np.broadcast_to(wq_c[None], (N_CORES,) + wq_c.shape),
        np.broadcast_to(wk_c[None], (N_CORES,) + wk_c.shape)], axis=-1))

    # vkb pack: [core, g, p, j, v 256]; wv separate fp8 [p, pc, d]
    vkb = _bf16(tr_chunks(vf))
    wv8 = _fp8(w_vs.T.reshape(32, 128, D).transpose(1, 0, 2), SWV)

    # wfc pack [pt, d-sub, dc, p-col]: wfcT_eff[dc*128+d, pt*512+p]
    wfcT_eff = _bf16((w_fc * ln_gamma[None, :]).T)     # [D, P] bf16
    wfc = np.ascontiguousarray(
        wfcT_eff.reshape(4, 128, NPT, 512).transpose(2, 1, 0, 3))
    colsum64 = np.asarray(wfcT_eff, np.float64).sum(axis=1)   # [D]
    colsum = _bf16(colsum64.reshape(4, 128).T)                # [128, 4]

    bias_fc = (w_fc @ ln_beta).astype(np.float32)
    veff = _bf16(vf + bias_fc[None, None, :])                 # [B, C, P] bf16
    vsum_all = np.asarray(veff, np.float64).sum(axis=2)       # [B, C]
    bng = np.ascontiguousarray(bn_gamma.reshape(C, 1))
    bnb = np.ascontiguousarray(bn_beta.reshape(C, 1))

    nc = _build()
    in_maps = []
    for i in range(N_CORES):
        bs = slice(BPC * i, BPC * (i + 1))
        in_maps.append({
            "qk8": qk8[i], "vkb": vkb[i], "wv8": wv8, "veff": veff[bs], "wfc": wfc,
            "colsum": colsum,
            "vsum": np.ascontiguousarray(
                vsum_all[bs].sum(axis=0).astype(np.float32).reshape(C, 1)),
            "bng": bng, "bnb": bnb,
        })
    res = run_bass_kernel_spmd(nc, in_maps, core_ids=list(range(N_CORES)))
    global LAST_RESULTS
    LAST_RESULTS = res
    out = np.concatenate([np.asarray(res.results[i]["out"])
                          for i in range(N_CORES)], axis=0)
    return out.reshape(B, C, HH, WW).astype(np.float32)
